# revision 65
# baseline (speedup 1.0000x reference)
"""Trainium2 Bass kernel for nn_DeterministicEgnnPolicy (EGNN message passing).

Strategy (per sharding hint): shard the 1024 independent 32-node graphs
across 8 NeuronCores (128 graphs/core). Dense all-pairs 32x32 edge blocks,
two graph-halves packed into 128 partitions with 64-feature blocks.

v2 pipeline: software-pipelined edge units with 4 single-buffer PSUM stage
pools (8 banks exactly), silu(c1) replaced by a host-folded quadratic
(pre-activations stay in [-1.1, 0.9]; LS quad fit), magg j-reduction on
GpSimd, Wc2-bias and quad constants folded into one fused DVE op in the
matrix phase, ps4 rows DMA'd directly to the s-matrix layout.

Graph indexing on a core: g = gb*4 + gm, gb in [0,32), gm in [0,4).
half = gb//16 (feature partitions 64*half..64*half+63).
node free index (per half): n' = gb_l*128 + gm*32 + i, gb_l = gb%16.
"""

import numpy as np

N_AGENTS = 32
BATCH = 1024
H = 64
L = 4
INV = 16
DEG = float(N_AGENTS - 1)
NCORES = 8
G_CORE = BATCH // NCORES          # 128 graphs per core
NGB = G_CORE // 4                 # 32 gb blocks per core
NGBL = NGB // 2                   # 16 per half
NNODE = NGBL * 128                # 2048 node free dim (per half)
NODES_CORE = G_CORE * N_AGENTS    # 4096

# Every other edge unit evaluates silu(c1-preact) by its quartic Taylor
# series (exact for the observed preact range [-1.1, 0.9] to ~1e-3);
# the rest use the exact ACT silu. This balances PE vs ACT engine load.
_QC1, _QC2, _QC4 = 0.5, 0.25, -1.0 / 48.0

_BUILD_CACHE = {}


def _silu_np(x):
    return x / (1.0 + np.exp(-x))


# ----------------------------------------------------------------------------
# Host-side packing (pure layout permutation / weight arrangement)
# ----------------------------------------------------------------------------

def _bd(w):
    """64x64 block-diagonal lhsT [128,128] from w [k,64]."""
    k = w.shape[0]
    out = np.zeros((128, 128), np.float32)
    out[0:k, 0:64] = w
    out[64:64 + k, 64:128] = w
    return out


def _bd_rep(wcol):
    """Replicating lhsT: out[64h+f, 64h+f'] = wcol[f] for all f'."""
    out = np.zeros((128, 128), np.float32)
    col = wcol.reshape(64, 1)
    out[0:64, 0:64] = np.repeat(col, 64, axis=1)
    out[64:128, 64:128] = np.repeat(col, 64, axis=1)
    return out


def _wnames():
    names = ["ident", "delta", "emb"]
    for l in range(L):
        for nm in ("Wi", "Wj", "Wsc", "We2", "Wc1", "Wc2", "Wv1", "Wv2",
                   "Wn1t", "Wn1b", "Wn1d", "Wn2", "Wlin", "Wsq", "Wsq2"):
            names.append(f"{nm}{l}")
    return names


def _bnames():
    names = []
    for l in range(L):
        for nm in ("be1", "be2", "bc1", "bv1", "bn1", "bn2", "bv2", "bc2",
                   "bcq", "wr", "we"):
            names.append(f"{nm}{l}")
    names.append("embb")
    return names


def _pack_weights(inp):
    """Build wpack [128, NW*128] and biaspack [128, NBIAS]."""
    tiles = {}

    def add(name, arr):
        t = np.zeros((128, 128), np.float32)
        t[:arr.shape[0], :arr.shape[1]] = arr
        tiles[name] = t

    add("ident", np.eye(128, dtype=np.float32))
    delta = np.zeros((4, 128), np.float32)
    for gm in range(4):
        delta[gm, gm * 32:(gm + 1) * 32] = 1.0
    add("delta", delta)

    emb = np.zeros((128, 128), np.float32)
    emb[0:INV, 0:64] = inp["emb_W"]
    emb[64:64 + INV, 64:128] = inp["emb_W"]
    add("emb", emb)

    for l in range(L):
        We1 = inp["We1"][l]          # [130, 64]
        add(f"Wi{l}", _bd(We1[0:64]))
        add(f"Wj{l}", _bd(We1[64:128]))
        wsc = np.zeros((4, 128), np.float32)
        wsc[0, 0:64] = We1[128]      # radial, half0
        wsc[1, 0:64] = We1[129]      # edge_attr, half0
        wsc[2, 64:128] = We1[128]
        wsc[3, 64:128] = We1[129]
        add(f"Wsc{l}", wsc)
        add(f"We2{l}", _bd(inp["We2"][l]))
        add(f"Wc1{l}", _bd(inp["Wc1"][l]))
        add(f"Wc2{l}", _bd_rep(inp["Wc2"][l][:, 0]))
        add(f"Wv1{l}", _bd(inp["Wv1"][l]))
        add(f"Wv2{l}", _bd_rep(inp["Wv2"][l][:, 0]))
        Wn1 = inp["Wn1"][l]          # [128, 64]
        add(f"Wn1t{l}", _bd(Wn1[0:64]))
        add(f"Wn1b{l}", _bd(Wn1[64:128]))
        add(f"Wn1d{l}", _bd(-Wn1[64:128]))
        add(f"Wn2{l}", _bd(inp["Wn2"][l]))
        # quartic c1-silu path: s = vlin.m + Wsq.(y^2) + Wsq2.(y^4) + const
        # silu(y+beta) expanded in y around beta = bc1[l] (beta = 0 here,
        # so the odd y^3 coefficient vanishes exactly)
        beta = inp["bc1"][l].reshape(-1)            # [64]
        wc2 = inp["Wc2"][l][:, 0]                   # [64]
        Bf = _QC1 + 2.0 * _QC2 * beta + 4.0 * _QC4 * beta ** 3
        Cf = _QC2 + 6.0 * _QC4 * beta ** 2
        vlin = inp["Wc1"][l] @ (wc2 * Bf)           # [64]
        add(f"Wlin{l}", _bd_rep(vlin))
        add(f"Wsq{l}", _bd_rep(wc2 * Cf))
        add(f"Wsq2{l}", _bd_rep(wc2 * np.full(64, _QC4, np.float32)))

    wnames = _wnames()
    wpack = np.concatenate([tiles[n] for n in wnames], axis=1)
    import ml_dtypes
    wpackb = np.concatenate([tiles[n] for n in _wbnames()], axis=1).astype(
        ml_dtypes.bfloat16)

    bias_cols = {}
    for l in range(L):
        for nm in ("be1", "be2", "bc1", "bv1", "bn1", "bn2"):
            bias_cols[f"{nm}{l}"] = np.tile(inp[nm][l].reshape(-1), 2)
        for nm in ("bv2", "bc2"):
            bias_cols[f"{nm}{l}"] = np.full(
                128, float(inp[nm][l].reshape(-1)[0]), np.float32)
        beta = inp["bc1"][l].reshape(-1)
        wc2 = inp["Wc2"][l][:, 0]
        const = float(np.sum(wc2 * (_QC1 * beta + _QC2 * beta ** 2
                                    + _QC4 * beta ** 4))
                      + inp["bc2"][l].reshape(-1)[0])
        bias_cols[f"bcq{l}"] = np.full(128, const, np.float32)
        bias_cols[f"wr{l}"] = np.tile(inp["We1"][l][128], 2)
        bias_cols[f"we{l}"] = np.tile(inp["We1"][l][129], 2)
    bias_cols["embb"] = np.tile(inp["emb_b"], 2)
    bnames = _bnames()
    biaspack = np.stack([bias_cols[n] for n in bnames], axis=1).astype(np.float32)
    return wpack, wpackb, biaspack


def _wbnames():
    names = []
    for l in range(L):
        for nm in ("We2", "Wc1", "Wc2", "Wlin", "Wsq", "Wsq2"):
            names.append(f"{nm}{l}")
    return names


def _arrange_inputs(obs_slice):
    """Per-core obs slice [4096, 20] -> invT [128, 2048], locvel [128, 128]."""
    obs3 = obs_slice.reshape(NGB, 128, 20)          # [gb, (gm,i), col]
    invT = np.zeros((128, NNODE), np.float32)
    inv_half0 = obs3[0:NGBL, :, 0:INV]              # [16, 128, 16]
    inv_half1 = obs3[NGBL:NGB, :, 0:INV]
    invT[0:INV, :] = np.transpose(inv_half0, (2, 0, 1)).reshape(INV, NNODE)
    invT[64:64 + INV, :] = np.transpose(inv_half1, (2, 0, 1)).reshape(INV, NNODE)
    locvel = np.ascontiguousarray(
        np.transpose(obs3[:, :, INV:INV + 4], (1, 0, 2)).reshape(128, NGB * 4)
    ).astype(np.float32)
    return invT, locvel


def _unarrange_output(outP):
    """outP [128, 64] -> [4096, 2] (n = gb*128 + p)."""
    return np.ascontiguousarray(
        outP.reshape(128, NGB, 2).transpose(1, 0, 2).reshape(NODES_CORE, 2)
    )


# ----------------------------------------------------------------------------
# Device kernel builder
# ----------------------------------------------------------------------------

def build(scale0, scale1, mean0, mean1):
    import concourse.bacc as bacc
    import concourse.tile as tile
    import concourse.mybir as mybir
    from contextlib import ExitStack

    F32 = mybir.dt.float32
    F32R = mybir.dt.float32r
    BF16 = mybir.dt.bfloat16
    AT = mybir.AluOpType
    ACTF = mybir.ActivationFunctionType

    def r32(ap):
        return ap.bitcast(F32R)

    nc = bacc.Bacc("TRN2", target_bir_lowering=False, debug=False)

    invT_d = nc.dram_tensor("invT", [128, NNODE], F32R, kind="ExternalInput")
    locvel_d = nc.dram_tensor("locvel", [128, NGB * 4], F32, kind="ExternalInput")
    NW = len(_wnames())
    wpack_d = nc.dram_tensor("wpack", [128, NW * 128], F32R, kind="ExternalInput")
    NWB = len(_wbnames())
    wpackb_d = nc.dram_tensor("wpackb", [128, NWB * 128], BF16, kind="ExternalInput")
    NBIAS = len(_bnames())
    bias_d = nc.dram_tensor("biaspack", [128, NBIAS], F32, kind="ExternalInput")
    out_d = nc.dram_tensor("out", [128, NGB * 2], F32, kind="ExternalOutput")

    widx = {n: i for i, n in enumerate(_wnames())}
    wbidx = {n: i for i, n in enumerate(_wbnames())}
    bidx = {n: i for i, n in enumerate(_bnames())}

    with tile.TileContext(nc) as tc, ExitStack() as ctx:
        st = ctx.enter_context(tc.tile_pool(name="static", bufs=1))
        eA = ctx.enter_context(tc.tile_pool(name="eA", bufs=2))   # m1s
        eM = ctx.enter_context(tc.tile_pool(name="eM", bufs=4))   # m_u
        eQ = ctx.enter_context(tc.tile_pool(name="eQ", bufs=2))   # c1t
        eR = ctx.enter_context(tc.tile_pool(name="eR", bufs=3))   # rsc
        eT = ctx.enter_context(tc.tile_pool(name="eT", bufs=2))   # t-chain
        eS = ctx.enter_context(tc.tile_pool(name="eS", bufs=2))   # ssb
        mx = ctx.enter_context(tc.tile_pool(name="mx", bufs=1))
        P1 = ctx.enter_context(tc.tile_pool(name="P1", bufs=1, space="PSUM"))
        P2 = ctx.enter_context(tc.tile_pool(name="P2", bufs=1, space="PSUM"))
        P3 = ctx.enter_context(tc.tile_pool(name="P3", bufs=1, space="PSUM"))
        P4 = ctx.enter_context(tc.tile_pool(name="P4", bufs=1, space="PSUM"))

        # ---- static loads ----
        wsb = st.tile([128, NW * 128], F32R)
        nc.sync.dma_start(wsb[:], wpack_d.ap())
        bsb = st.tile([128, NBIAS], F32)
        nc.sync.dma_start(bsb[:], bias_d.ap())
        locvel = st.tile([128, NGB * 4], F32)
        nc.sync.dma_start(locvel[:], locvel_d.ap())

        def W(name):
            return wsb[:, widx[name] * 128:(widx[name] + 1) * 128]

        def Bia(name):
            return bsb[:, bidx[name]:bidx[name] + 1]

        ident = W("ident").bitcast(F32)
        delta4 = W("delta").bitcast(F32)[0:4, :]

        # ---- persistent state ----
        hA = st.tile([128, NNODE], F32R)
        hB = st.tile([128, NNODE], F32R)
        # hB doubles as the invT staging buffer (consumed by the embedding
        # before layer 0's h-update writes h_next into it)
        invT = hB
        nc.sync.dma_start(invT[:], invT_d.ap())
        magg = st.tile([128, NNODE], F32R)
        mdiag = st.tile([128, NNODE], F32R)
        smat = st.tile([128, 1024], F32)
        rad = st.tile([128, 1024], F32R)
        ea = st.tile([128, 1024], F32R)
        dx = st.tile([128, 1024], F32)
        dy = st.tile([128, 1024], F32)
        locx = st.tile([128, NGB], F32)
        locy = st.tile([128, NGB], F32)
        velx = st.tile([128, NGB], F32)
        vely = st.tile([128, NGB], F32)
        phiP = st.tile([128, NGB], F32)
        hv1 = st.tile([128, 1024], F32R)
        phirep = st.tile([128, NNODE], F32)
        lxT = st.tile([32, 128], F32)
        lyT = st.tile([32, 128], F32)
        T4x = st.tile([4, 1024], F32)
        T4y = st.tile([4, 1024], F32)
        tm = st.tile([128, 1024], F32)
        outP = st.tile([128, NGB * 2], F32)
        A8 = st.tile([128, NNODE], BF16)
        B8 = st.tile([128, NNODE], BF16)
        rad8 = st.tile([128, 1024], BF16)
        ea8 = st.tile([128, 1024], BF16)

        lv = locvel[:].rearrange("p (gb c) -> p gb c", c=4)
        nc.vector.tensor_copy(locx[:], lv[:, :, 0])
        nc.vector.tensor_copy(locy[:], lv[:, :, 1])
        nc.vector.tensor_copy(velx[:], lv[:, :, 2])
        nc.vector.tensor_copy(vely[:], lv[:, :, 3])

        def heat(lhsT_ap, rhs_ap, n=14):
            hp = P4.tile([128, 1024], F32, tag="stage")
            for _ in range(n):
                nc.tensor.matmul(hp[:, 0:512], lhsT_ap, rhs_ap, start=True, stop=True)

        # ---- embedding: h0 = inv @ emb_W + emb_b ----
        heat(r32(W("emb")), r32(invT[:, 0:512]))
        for u in range(NNODE // 1024):
            pse = (P1 if u == 0 else P2).tile([128, 1024], F32, tag="stage")
            for k in range(2):
                nc.tensor.matmul(pse[:, k * 512:(k + 1) * 512], r32(W("emb")),
                                 r32(invT[:, u * 1024 + k * 512:u * 1024 + (k + 1) * 512]),
                                 start=True, stop=True)
            nc.vector.tensor_scalar_add(hA[:, u * 1024:(u + 1) * 1024], pse[:], Bia("embb"))

        def radial_part(first):
            """Compute lxT/lyT, T4s, dx, dy, rad from current locx/locy."""
            for (lP, lT) in ((locx, lxT), (locy, lyT)):
                pst = P4.tile([128, 1024], F32, tag="stage")
                nc.tensor.transpose(pst[0:32, 0:128], lP[:], ident)
                nc.vector.tensor_copy(lT[:], pst[0:32, 0:128])
            for (lT, T4) in ((lxT, T4x), (lyT, T4y)):
                for gm in range(4):
                    nc.sync.dma_start(
                        T4[gm:gm + 1, :].rearrange("p (gb j) -> p gb j", j=32),
                        lT[:, gm * 32:(gm + 1) * 32])
            for (T4, lP, dT) in ((T4x, locx, dx), (T4y, locy, dy)):
                pss = P1.tile([128, 1024], F32, tag="stage")
                for k in range(2):
                    nc.tensor.matmul(pss[:, k * 512:(k + 1) * 512], delta4,
                                     T4[:, k * 512:(k + 1) * 512],
                                     start=True, stop=True)
                bc = lP[:].unsqueeze(2).broadcast_to([128, NGB, 32])
                nc.vector.tensor_tensor(
                    dT[:].rearrange("p (gb j) -> p gb j", j=32), bc,
                    pss[:].rearrange("p (gb j) -> p gb j", j=32), op=AT.subtract)
            t2 = mx.tile([128, 1024], F32, tag="mx_w")
            nc.vector.tensor_tensor(rad[:], dx[:], dx[:], op=AT.mult)
            nc.vector.tensor_tensor(t2[:], dy[:], dy[:], op=AT.mult)
            nc.vector.tensor_tensor(rad[:], rad[:], t2[:], op=AT.add)
            if first:
                nc.vector.tensor_copy(ea[:], rad[:])

        radial_part(first=True)

        # per-layer edge unit emitters ---------------------------------------
        # edge stage 1 off the (clock-throttled) PE: A = Wi.h, B = Wj.h are
        # per-node; per-edge pre-act = A_i + B_j + wr*rad + we*ea built from
        # DMA-replicated tiles with two fused DVE ops in bf16 (4x mode).
        def emit_tAB(l, i):
            gb_l, gmp, u = i >> 2, (i >> 1) & 1, i & 1
            nb = gb_l * 128 + gmp * 64 + u * 32
            tAB = eT.tile([128, 1024], BF16, tag="tAB")
            nc.gpsimd.tensor_tensor(
                tAB[:].rearrange("p (i j) -> p i j", j=32),
                A8[:, nb:nb + 32].unsqueeze(2).broadcast_to([128, 32, 32]),
                B8[:, nb:nb + 32].unsqueeze(1).broadcast_to([128, 32, 32]),
                op=AT.add)
            return tAB

        def emit_S1(l, h, i, tAB):
            gb_l, gmp, u = i >> 2, (i >> 1) & 1, i & 1
            if u == 0:
                p0 = gmp * 64
                rsc = eR.tile([4, 2048], F32R, tag="rsc")
                for (row, src) in ((0, rad), (1, ea)):
                    nc.sync.dma_start(
                        rsc[row:row + 1, :].rearrange(
                            "p (a b c) -> p a b c", a=2, b=32, c=32),
                        src[p0:p0 + 64, gb_l * 32:(gb_l + 1) * 32])
                    nc.sync.dma_start(
                        rsc[row + 2:row + 3, :].rearrange(
                            "p (a b c) -> p a b c", a=2, b=32, c=32),
                        src[p0:p0 + 64, (gb_l + 16) * 32:(gb_l + 17) * 32])
                emit_S1.rsc = rsc
            rsc = emit_S1.rsc
            ps1 = P1.tile([128, 1024], F32, tag="stage")
            for k in range(2):
                ksl = slice(k * 512, (k + 1) * 512)
                nc.tensor.matmul(ps1[:, ksl], r32(W(f"Wsc{l}")[0:4, :]),
                                 r32(rsc[:, u * 1024 + k * 512:u * 1024 + (k + 1) * 512]),
                                 start=True, stop=True)
            pre1 = eT.tile([128, 1024], BF16, tag="pre1")
            nc.vector.tensor_tensor(pre1[:], tAB[:], ps1[:], op=AT.add)
            return pre1

        def emit_A1(l, st_u):
            m1s = eA.tile([128, 1024], F32R, tag="m1s")
            nc.scalar.activation(m1s[:], st_u["pre1"][:], ACTF.Silu, bias=Bia(f"be1{l}"))
            st_u["m1s"] = m1s

        def emit_S2(l, st_u):
            ps2 = P2.tile([128, 1024], F32, tag="stage")
            m1s = st_u["m1s"]
            for k in range(2):
                ksl = slice(k * 512, (k + 1) * 512)
                nc.tensor.matmul(ps2[:, ksl], r32(W(f"We2{l}")),
                                 r32(m1s[:, ksl]), start=True, stop=True)
            st_u["ps2"] = ps2

        def emit_A2(l, st_u):
            m_u = eM.tile([128, 1024], F32R, tag="m_u")
            nc.scalar.activation(m_u[:], st_u["ps2"][:], ACTF.Silu, bias=Bia(f"be2{l}"))
            st_u["m_u"] = m_u

        def emit_red(l, st_u):
            nb = st_u["nb"]
            m_u = st_u["m_u"]
            with nc.allow_low_precision(reason="fp32r magg"):
                nc.vector.tensor_reduce(
                    magg[:, nb:nb + 32],
                    m_u[:].rearrange("p (i j) -> p i j", j=32),
                    axis=mybir.AxisListType.X, op=AT.add)
            nc.gpsimd.tensor_copy(mdiag[:, nb:nb + 32], m_u[:, 0:1024:33])

        def emit_S3(l, st_u):
            ps3 = P3.tile([128, 1024], F32, tag="stage")
            m_u = st_u["m_u"]
            for k in range(2):
                ksl = slice(k * 512, (k + 1) * 512)
                nc.tensor.matmul(ps3[:, ksl], r32(W(f"Wc1{l}")),
                                 r32(m_u[:, ksl]), start=True, stop=True)
            st_u["ps3"] = ps3

        def emit_sq(l, st_u):
            if st_u["quad"]:
                yt = eQ.tile([128, 1024], BF16, tag="yt")
                nc.vector.tensor_copy(yt[:], st_u["ps3"][:])
                sq = eQ.tile([128, 1024], BF16, tag="sq")
                nc.gpsimd.tensor_tensor(sq[:], yt[:], yt[:], op=AT.mult)
                sq2 = eQ.tile([128, 1024], BF16, tag="sq2")
                nc.gpsimd.tensor_tensor(sq2[:], sq[:], sq[:], op=AT.mult)
                st_u["sq"], st_u["sq2"] = sq, sq2
            else:
                c1t = eQ.tile([128, 1024], F32R, tag="c1t")
                nc.scalar.activation(c1t[:], st_u["ps3"][:], ACTF.Silu,
                                     bias=Bia(f"bc1{l}"))
                st_u["c1t"] = c1t

        def emit_S4(l, st_u):
            ps4 = P4.tile([128, 1024], F32, tag="stage")
            c1t = st_u["c1t"]
            for k in range(2):
                ksl = slice(k * 512, (k + 1) * 512)
                nc.tensor.matmul(ps4[:, ksl], r32(W(f"Wc2{l}")),
                                 r32(c1t[:, ksl]), start=True, stop=True)
            st_u["ps4"] = ps4

        def emit_out(l, st_u):
            # s + const (replicated rows) -> matrix layout via SBUF hop
            i = st_u["i"]
            gb_l, gmg = i >> 2, i & 3
            ps4 = st_u["ps4"]
            ssb = eS.tile([128, 1024], F32, tag="ssb")
            nc.vector.tensor_copy(ssb[:], ps4[:])
            pg = gmg * 32
            nc.sync.dma_start(
                smat[pg:pg + 32, gb_l * 32:(gb_l + 1) * 32],
                ssb[0:1, :].rearrange("p (i j) -> p i j", j=32))
            nc.sync.dma_start(
                smat[pg:pg + 32, (gb_l + 16) * 32:(gb_l + 17) * 32],
                ssb[64:65, :].rearrange("p (i j) -> p i j", j=32))

        def emit_hupd(l, h, h_next, u):
            sl = slice(u * 1024, (u + 1) * 1024)
            psh = P3.tile([128, 1024], F32, tag="stage")
            for k in range(2):
                ksl = slice(u * 1024 + k * 512, u * 1024 + (k + 1) * 512)
                osl = slice(k * 512, (k + 1) * 512)
                nc.tensor.matmul(psh[:, osl], r32(W(f"Wn1t{l}")),
                                 r32(h[:, ksl]), start=True, stop=False)
                nc.tensor.matmul(psh[:, osl], r32(W(f"Wn1b{l}")),
                                 r32(magg[:, ksl]), start=False, stop=False)
                nc.tensor.matmul(psh[:, osl], r32(W(f"Wn1d{l}")),
                                 r32(mdiag[:, ksl]), start=False, stop=True)
            hn1 = eA.tile([128, 1024], F32R, tag="m1s")
            nc.scalar.activation(hn1[:], psh[:], ACTF.Silu, bias=Bia(f"bn1{l}"))
            psh2 = P4.tile([128, 1024], F32, tag="stage")
            for k in range(2):
                osl = slice(k * 512, (k + 1) * 512)
                nc.tensor.matmul(psh2[:, osl], r32(W(f"Wn2{l}")),
                                 r32(hn1[:, osl]), start=True, stop=True)
            nc.vector.scalar_tensor_tensor(
                h_next[:, sl], psh2[:], Bia(f"bn2{l}"), h[:, sl],
                op0=AT.add, op1=AT.add)

        for l in range(L):
            h = hA if l % 2 == 0 else hB
            h_next = hB if l % 2 == 0 else hA

            # ---- tm = 1/(1+sqrt(rad)) for this layer's matrix phase ----
            nc.scalar.activation(tm[:], rad[:], ACTF.Sqrt)
            nc.vector.tensor_scalar_add(tm[:], tm[:], 1.0)
            nc.vector.reciprocal(tm[:], tm[:])

            # ---- bf16 copies of rad/ea + per-node A = Wi.h, B = Wj.h ----
            nc.vector.tensor_copy(rad8[:], rad[:])
            if l == 0:
                nc.vector.tensor_copy(ea8[:], ea[:])
            for (wn, dst) in ((f"Wi{l}", A8), (f"Wj{l}", B8)):
                for u in range(NNODE // 1024):
                    psab = P1.tile([128, 1024], F32, tag="stage")
                    for k in range(2):
                        ksl = slice(u * 1024 + k * 512, u * 1024 + (k + 1) * 512)
                        nc.tensor.matmul(psab[:, k * 512:(k + 1) * 512],
                                         r32(W(wn)), r32(h[:, ksl]),
                                         start=True, stop=True)
                    nc.vector.tensor_copy(dst[:, u * 1024:(u + 1) * 1024], psab[:])

            # ---- node phase: phi = silu(h@Wv1+bv1)@Wv2 + bv2 -> phiP ----
            for u in range(NNODE // 1024):
                sl = slice(u * 1024, (u + 1) * 1024)
                psv = P1.tile([128, 1024], F32, tag="stage")
                for k in range(2):
                    ksl = slice(u * 1024 + k * 512, u * 1024 + (k + 1) * 512)
                    nc.tensor.matmul(psv[:, k * 512:(k + 1) * 512],
                                     r32(W(f"Wv1{l}")), r32(h[:, ksl]),
                                     start=True, stop=True)
                nc.scalar.activation(hv1[:], psv[:], ACTF.Silu, bias=Bia(f"bv1{l}"))
                psv2 = P2.tile([128, 1024], F32, tag="stage")
                for k in range(2):
                    nc.tensor.matmul(psv2[:, k * 512:(k + 1) * 512],
                                     r32(W(f"Wv2{l}")),
                                     r32(hv1[:, k * 512:(k + 1) * 512]),
                                     start=True, stop=True)
                nc.vector.tensor_scalar_add(phirep[:, sl], psv2[:], Bia(f"bv2{l}"))
            for c in range(NGBL):
                pst = P3.tile([128, 1024], F32, tag="stage")
                nc.tensor.transpose(pst[:, 0:128], phirep[:, c * 128:(c + 1) * 128], ident)
                nc.vector.tensor_copy(phiP[:, c:c + 1], pst[:, 0:1])
                nc.vector.tensor_copy(phiP[:, c + NGBL:c + NGBL + 1], pst[:, 64:65])

            # ---- software-pipelined edge units ----
            NU = 64
            stq = {}
            emitted_h0 = False

            def pipe_step(i):
                # PE emission order: S1(i), S2(i-1), S3(i-2), S4(i-3)
                if i == 0:
                    stq[0] = {"i": 0, "nb": 0, "quad": False,
                              "tAB": emit_tAB(l, 0)}
                if i + 1 < NU:
                    stq[i + 1] = {"i": i + 1,
                                  "nb": ((i + 1) >> 2) * 128 + ((i + 1) & 3) * 32,
                                  "quad": False, "tAB": emit_tAB(l, i + 1)}
                if i < NU:
                    stq[i]["pre1"] = emit_S1(l, h, i, stq[i]["tAB"])
                    emit_A1(l, stq[i])
                if 0 <= i - 1 < NU:
                    emit_S2(l, stq[i - 1])
                    emit_A2(l, stq[i - 1])
                if 0 <= i - 2 < NU:
                    emit_S3(l, stq[i - 2])
                    emit_red(l, stq[i - 2])
                    emit_sq(l, stq[i - 2])
                if 0 <= i - 3 < NU:
                    emit_S4(l, stq[i - 3])
                    emit_out(l, stq[i - 3])
                    del stq[i - 3]

            for i in range(NU + 4):
                pipe_step(i)
                if i == NU // 2 + 3:
                    emit_hupd(l, h, h_next, 0)
            emit_hupd(l, h, h_next, 1)

            # ---- matrix phase: um, agg, vel/loc update; then radial(l+1) ----
            um = mx.tile([128, 1024], F32, tag="mx_um")
            nc.vector.scalar_tensor_tensor(um[:], smat[:], Bia(f"bc2{l}"),
                                           tm[:], op0=AT.add, op1=AT.mult)
            for (dT, agg_out) in ((dx, "ax"), (dy, "ay")):
                w_ = mx.tile([128, 1024], F32, tag="mx_w")
                nc.vector.tensor_tensor(w_[:], um[:], dT[:], op=AT.mult)
                ag = mx.tile([128, NGB], F32, tag="mx_" + agg_out)
                nc.vector.tensor_reduce(
                    ag[:], w_[:].rearrange("p (gb j) -> p gb j", j=32),
                    axis=mybir.AxisListType.X, op=AT.add)
                vP = velx if agg_out == "ax" else vely
                tmp = mx.tile([128, NGB], F32, tag="mx_tmp")
                nc.vector.tensor_tensor(tmp[:], phiP[:], vP[:], op=AT.mult)
                nc.vector.scalar_tensor_tensor(vP[:], ag[:], 1.0 / DEG, tmp[:],
                                               op0=AT.mult, op1=AT.add)
            nc.vector.tensor_tensor(locx[:], locx[:], velx[:], op=AT.add)
            nc.vector.tensor_tensor(locy[:], locy[:], vely[:], op=AT.add)
            if l < L - 1:
                radial_part(first=False)

        # ---- output: outP interleaved (gb, c) ----
        ov = outP[:].rearrange("p (gb c) -> p gb c", c=2)
        nc.vector.tensor_scalar(ov[:, :, 0], velx[:], scale0, mean0,
                                op0=AT.mult, op1=AT.add)
        nc.vector.tensor_scalar(ov[:, :, 1], vely[:], scale1, mean1,
                                op0=AT.mult, op1=AT.add)
        nc.sync.dma_start(out_d.ap(), outP[:])

    nc.compile()
    return nc


# ----------------------------------------------------------------------------
# Entry point
# ----------------------------------------------------------------------------

def kernel(**inputs):
    import concourse.mybir  # noqa: F401  (ensure env importable)
    from concourse.bass_utils import run_bass_kernel_spmd

    inp = {k: np.asarray(v) for k, v in inputs.items()}
    obs = inp["obs"].astype(np.float32)
    scale = np.asarray(inp["scale"], np.float32)
    mean = np.asarray(inp["mean"], np.float32)

    key = (float(scale[0]), float(scale[1]), float(mean[0]), float(mean[1]))
    if key not in _BUILD_CACHE:
        _BUILD_CACHE[key] = build(*key)
    nc = _BUILD_CACHE[key]

    wpack, wpackb, biaspack = _pack_weights(inp)
    in_maps = []
    for c in range(NCORES):
        invT, locvel = _arrange_inputs(obs[c * NODES_CORE:(c + 1) * NODES_CORE])
        in_maps.append({"invT": invT, "locvel": locvel, "wpack": wpack,
                        "wpackb": wpackb, "biaspack": biaspack})
    res = run_bass_kernel_spmd(nc, in_maps, list(range(NCORES)))
    outs = [_unarrange_output(res.results[c]["out"]) for c in range(NCORES)]
    return np.concatenate(outs, axis=0)
